# revision 1
# baseline (speedup 1.0000x reference)
"""Trainium2 Bass kernel for an AQT quantized Dense layer — packed-run variant.

Same math as kernel.py; differences:
  - host ships x as [NSB, P, KC, SB] and receives y as [NSB, P, FC, F]
    (per-partition DMA runs of 8KB instead of 2KB -> ~4x fewer DMA packets,
    less per-packet metadata overhead, denser HBM access)
  - weight prep runs entirely on DVE (no ACT) so the one-time ACT table load
    does not sit on the critical path to the first matmul
"""

import numpy as np

B, D, F = 131072, 512, 512
NCORES = 8
BS = B // NCORES           # rows per core
P = 128                    # partitions
KC = D // P                # contraction chunks
FC = F // P                # f chunks
SB = 512                   # superblock: b-rows per block
NSB = BS // SB             # superblocks per core

MAGIC = float(np.float32(1.5 * 2**23))            # 12582912.0
A_SCALE = float(np.float32(127.0 / 6.0))
INV_K = float(np.float32(6.0 / (127.0 * 127.0)))  # inv_scale = w_bound * INV_K
EPS = 1e-6

_NC_CACHE = {}


def _build_nc():
    import concourse.bacc as bacc
    import concourse.mybir as mybir
    import concourse.tile as tile
    from concourse.masks import make_identity

    f32 = mybir.dt.float32
    bf16 = mybir.dt.bfloat16

    nc = bacc.Bacc("TRN2", target_bir_lowering=False, debug=False,
                   enable_asserts=False)
    x_t = nc.dram_tensor("xt", [NSB, P, KC, SB], f32, kind="ExternalInput")
    k_t = nc.dram_tensor("kt", [F, D], f32, kind="ExternalInput")
    y_t = nc.dram_tensor("out", [NSB, P, FC, F], f32, kind="ExternalOutput")
    x_ap, k_ap, y_ap = x_t.ap(), k_t.ap(), y_t.ap()

    with tile.TileContext(nc) as tc:
        from contextlib import ExitStack
        with ExitStack() as ctx:
            const = ctx.enter_context(tc.tile_pool(name="const", bufs=1))
            wpool = ctx.enter_context(tc.tile_pool(name="wdeq", bufs=1))

            # main-loop pools open BEFORE the weight-prep pool (stack
            # allocator: prep scratch lands above, release doesn't overlap)
            xin = ctx.enter_context(tc.tile_pool(name="xin", bufs=6))
            tqp = ctx.enter_context(tc.tile_pool(name="tqp", bufs=2))
            xq = ctx.enter_context(tc.tile_pool(name="xq", bufs=6))
            yout = ctx.enter_context(tc.tile_pool(name="yout", bufs=6))
            mmps = ctx.enter_context(tc.tile_pool(name="mmps", bufs=7,
                                                  space="PSUM"))

            ident16 = const.tile([P, P], bf16, tag="ident16")
            make_identity(nc, ident16)

            # ---------------- weight prep (one-time, DVE only) ----------
            wdeq = []
            with tc.tile_pool(name="wprep", bufs=1) as wp, \
                 tc.tile_pool(name="wps", bufs=1, space="PSUM") as wps:
                wdT = []
                for j in range(FC):
                    kT = wp.tile([P, D], f32, tag=f"kT{j}")
                    nc.gpsimd.dma_start(out=kT, in_=k_ap[j * P:(j + 1) * P, :])
                    wb = wp.tile([P, 1], f32, tag=f"wb{j}")
                    nc.vector.tensor_reduce(wb, kT, axis=mybir.AxisListType.X,
                                            op=mybir.AluOpType.max,
                                            apply_absolute_value=True)
                    wbm = wp.tile([P, 1], f32, tag=f"wbm{j}")
                    nc.vector.tensor_scalar_max(wbm, wb, EPS)
                    rec = wp.tile([P, 1], f32, tag=f"rec{j}")
                    nc.vector.reciprocal(rec, wbm)
                    wsc = wp.tile([P, 1], f32, tag=f"wsc{j}")
                    nc.vector.tensor_scalar_mul(wsc, rec, 127.0)
                    inv = wp.tile([P, 1], f32, tag=f"inv{j}")
                    nc.vector.tensor_scalar_mul(inv, wbm, INV_K)
                    # tw = kT * w_scale + MAGIC   (DVE, per-partition scale)
                    tw = wp.tile([P, D], f32, tag=f"tw{j}")
                    nc.vector.tensor_scalar(tw, kT, wsc, MAGIC,
                                            op0=mybir.AluOpType.mult,
                                            op1=mybir.AluOpType.add)
                    # w_deqT = (tw - MAGIC) * inv_scale   -> bf16  [128_f, D]
                    wt = wp.tile([P, D], bf16, tag=f"wdT{j}")
                    nc.vector.tensor_scalar(wt, tw, MAGIC, inv,
                                            op0=mybir.AluOpType.subtract,
                                            op1=mybir.AluOpType.mult)
                    wdT.append(wt)
                # transpose back to natural layout w_deq[i] = [128_d, F] bf16
                for i in range(KC):
                    ps = wps.tile([P, F], bf16, tag="wdps")
                    for j in range(FC):
                        nc.tensor.transpose(ps[:, j * P:(j + 1) * P],
                                            wdT[j][:, i * P:(i + 1) * P],
                                            ident16)
                    wd = wpool.tile([P, F], bf16, tag=f"wdeq{i}")
                    nc.vector.tensor_copy(wd, ps)
                    wdeq.append(wd)

            # ---------------- main loop ----------------
            for s in range(NSB):
                # one 1MB load, fully contiguous (8KB per partition)
                xf = xin.tile([P, KC, SB], f32, tag="xf")
                nc.sync.dma_start(out=xf, in_=x_ap[s])
                # t = x*a_scale + MAGIC ; x_qT = t - MAGIC -> bf16
                tq = tqp.tile([P, KC, SB], f32, tag="tq")
                nc.vector.tensor_scalar(tq, xf, A_SCALE, MAGIC,
                                        op0=mybir.AluOpType.mult,
                                        op1=mybir.AluOpType.add)
                xqT = xq.tile([P, KC, SB], bf16, tag="xqT")
                nc.vector.tensor_scalar(xqT, tq, MAGIC, None,
                                        op0=mybir.AluOpType.subtract)
                # matmuls: y_tiled[s, p, j, f] = y[b0+128j+p, f]
                yf = yout.tile([P, FC, F], f32, tag="yf")
                for j in range(SB // P):
                    yp = mmps.tile([P, F], f32, tag="yp")
                    for k in range(KC):
                        nc.tensor.matmul(yp,
                                         xqT[:, k, j * P:(j + 1) * P],
                                         wdeq[k],
                                         start=(k == 0), stop=(k == KC - 1))
                    nc.scalar.copy(yf[:, j, :], yp)
                # one 1MB store, fully contiguous (8KB per partition)
                with tc.high_priority():
                    nc.scalar.dma_start(out=y_ap[s], in_=yf)

    nc.compile()
    return nc


def _get_nc():
    if "nc" not in _NC_CACHE:
        _NC_CACHE["nc"] = _build_nc()
    return _NC_CACHE["nc"]


def kernel(**inputs):
    from concourse.bass_utils import run_bass_kernel_spmd

    x = np.asarray(inputs["x"], dtype=np.float32)
    kern = np.asarray(inputs["kernel"], dtype=np.float32)

    kT = np.ascontiguousarray(kern.T)
    # packed layout: [NSB, P, KC, SB]; xtile[s, p, c, b] = x[s*SB+b, c*P+p]
    shards = [np.ascontiguousarray(
                  x[i * BS:(i + 1) * BS].reshape(NSB, SB, KC, P)
                  .transpose(0, 3, 2, 1))
              for i in range(NCORES)]

    nc = _get_nc()
    in_maps = [{"xt": s, "kt": kT} for s in shards]
    res = run_bass_kernel_spmd(nc, in_maps, core_ids=list(range(NCORES)))
    # un-tile: y[b0+128j+p, f] = y_tiled[s, p, j, f]
    out = np.concatenate(
        [r["out"].transpose(0, 2, 1, 3).reshape(BS, F) for r in res.results],
        axis=0)
    out = np.ascontiguousarray(out)

    bias = inputs.get("bias")
    if bias is not None and np.any(np.asarray(bias)):
        out = out + np.asarray(bias, dtype=np.float32)[None, :]
    return out



# revision 2
# speedup vs baseline: 1.2059x; 1.2059x over previous
"""Trainium2 Bass kernel for an AQT quantized Dense layer.

Host pre-quantizes x to integer values (exact AQT rounding) and ships them
as bf16 (integers <= 127 are exact in bf16), halving the x DMA traffic and
removing all device-side quantize ops. Output is written as bf16 (the
dequantized values are small; bf16 rounding is ~0.4% rel, far inside the
tolerance), halving the y DMA traffic. The device is then a pure
bf16-matmul pipeline: DMA in -> PE matmul -> PSUM->SBUF bf16 copy
(alternating ACT/DVE) -> DMA out, bound by the TensorEngine.
"""

import numpy as np

B, D, F = 131072, 512, 512
NCORES = 8
BS = B // NCORES           # rows per core
P = 128                    # partitions
KC = D // P                # contraction chunks
SB = 1024                  # superblock: b-rows per block
NSB = BS // SB             # superblocks per core
JC = SB // P               # b-chunks of 128 rows per superblock

MAGIC = float(np.float32(1.5 * 2**23))            # 12582912.0
A_SCALE = float(np.float32(127.0 / 6.0))
INV_K = float(np.float32(6.0 / (127.0 * 127.0)))  # inv_scale = w_bound * INV_K
EPS = 1e-6

_NC_CACHE = {}


def _build_nc():
    import concourse.bacc as bacc
    import concourse.mybir as mybir
    import concourse.tile as tile
    from concourse.masks import make_identity

    f32 = mybir.dt.float32
    bf16 = mybir.dt.bfloat16

    nc = bacc.Bacc("TRN2", target_bir_lowering=False, debug=False,
                   enable_asserts=False)
    x_t = nc.dram_tensor("xt", [NSB, P, KC, SB], bf16, kind="ExternalInput")
    k_t = nc.dram_tensor("kt", [F, D], f32, kind="ExternalInput")
    y_t = nc.dram_tensor("out", [NSB, P, JC, F], bf16, kind="ExternalOutput")
    x_ap, k_ap, y_ap = x_t.ap(), k_t.ap(), y_t.ap()

    with tile.TileContext(nc) as tc:
        from contextlib import ExitStack
        with ExitStack() as ctx:
            const = ctx.enter_context(tc.tile_pool(name="const", bufs=1))
            wpool = ctx.enter_context(tc.tile_pool(name="wdeq", bufs=1))

            # main-loop pools open BEFORE the weight-prep pool (stack
            # allocator: prep scratch lands above, release doesn't overlap)
            xin = ctx.enter_context(tc.tile_pool(name="xin", bufs=4))
            yout = ctx.enter_context(tc.tile_pool(name="yout", bufs=4))
            mmps = ctx.enter_context(tc.tile_pool(name="mmps", bufs=7,
                                                  space="PSUM"))

            ident16 = const.tile([P, P], bf16, tag="ident16")
            make_identity(nc, ident16)

            # ---------------- weight prep (one-time, DVE only) ----------
            wdeq = []
            with tc.tile_pool(name="wprep", bufs=1) as wp, \
                 tc.tile_pool(name="wps", bufs=1, space="PSUM") as wps:
                wdT = []
                for j in range(F // P):
                    kT = wp.tile([P, D], f32, tag=f"kT{j}")
                    nc.gpsimd.dma_start(out=kT, in_=k_ap[j * P:(j + 1) * P, :])
                    wb = wp.tile([P, 1], f32, tag=f"wb{j}")
                    nc.vector.tensor_reduce(wb, kT, axis=mybir.AxisListType.X,
                                            op=mybir.AluOpType.max,
                                            apply_absolute_value=True)
                    wbm = wp.tile([P, 1], f32, tag=f"wbm{j}")
                    nc.vector.tensor_scalar_max(wbm, wb, EPS)
                    rec = wp.tile([P, 1], f32, tag=f"rec{j}")
                    nc.vector.reciprocal(rec, wbm)
                    wsc = wp.tile([P, 1], f32, tag=f"wsc{j}")
                    nc.vector.tensor_scalar_mul(wsc, rec, 127.0)
                    inv = wp.tile([P, 1], f32, tag=f"inv{j}")
                    nc.vector.tensor_scalar_mul(inv, wbm, INV_K)
                    # tw = kT * w_scale + MAGIC   (DVE, per-partition scale)
                    tw = wp.tile([P, D], f32, tag=f"tw{j}")
                    nc.vector.tensor_scalar(tw, kT, wsc, MAGIC,
                                            op0=mybir.AluOpType.mult,
                                            op1=mybir.AluOpType.add)
                    # w_deqT = (tw - MAGIC) * inv_scale   -> bf16  [128_f, D]
                    wt = wp.tile([P, D], bf16, tag=f"wdT{j}")
                    nc.vector.tensor_scalar(wt, tw, MAGIC, inv,
                                            op0=mybir.AluOpType.subtract,
                                            op1=mybir.AluOpType.mult)
                    wdT.append(wt)
                # transpose back to natural layout w_deq[i] = [128_d, F] bf16
                for i in range(KC):
                    ps = wps.tile([P, F], bf16, tag="wdps")
                    for j in range(F // P):
                        nc.tensor.transpose(ps[:, j * P:(j + 1) * P],
                                            wdT[j][:, i * P:(i + 1) * P],
                                            ident16)
                    wd = wpool.tile([P, F], bf16, tag=f"wdeq{i}")
                    nc.vector.tensor_copy(wd, ps)
                    wdeq.append(wd)

            # ---------------- main loop ----------------
            for s in range(NSB):
                # one 1MB load, fully contiguous (8KB per partition)
                xf = xin.tile([P, KC, SB], bf16, tag="xf")
                nc.sync.dma_start(out=xf, in_=x_ap[s])
                yf = yout.tile([P, JC, F], bf16, tag="yf")
                for j in range(JC):
                    yp = mmps.tile([P, F], f32, tag="yp")
                    for k in range(KC):
                        nc.tensor.matmul(yp,
                                         xf[:, k, j * P:(j + 1) * P],
                                         wdeq[k],
                                         start=(k == 0), stop=(k == KC - 1))
                    # PSUM -> SBUF bf16, alternating engines to stay off
                    # the PE critical path
                    if j % 2 == 0:
                        nc.scalar.copy(yf[:, j, :], yp)
                    else:
                        nc.vector.tensor_copy(yf[:, j, :], yp)
                # one 1MB store, fully contiguous (8KB per partition)
                with tc.high_priority():
                    nc.gpsimd.dma_start(out=y_ap[s], in_=yf)

    nc.compile()
    return nc


def _get_nc():
    if "nc" not in _NC_CACHE:
        _NC_CACHE["nc"] = _build_nc()
    return _NC_CACHE["nc"]


def kernel(**inputs):
    import ml_dtypes
    from concourse.bass_utils import run_bass_kernel_spmd

    x = np.asarray(inputs["x"], dtype=np.float32)
    kern = np.asarray(inputs["kernel"], dtype=np.float32)

    kT = np.ascontiguousarray(kern.T)
    # exact AQT activation quantization on host; integer values <= 127 are
    # exactly representable in bf16
    xq = np.clip(np.rint(x * np.float32(A_SCALE)), -127.0, 127.0)
    xb = xq.astype(ml_dtypes.bfloat16)
    # packed layout: [NSB, P, KC, SB]; xtile[s, p, c, b] = x[s*SB+b, c*P+p]
    shards = [np.ascontiguousarray(
                  xb[i * BS:(i + 1) * BS].reshape(NSB, SB, KC, P)
                  .transpose(0, 3, 2, 1))
              for i in range(NCORES)]

    nc = _get_nc()
    in_maps = [{"xt": s, "kt": kT} for s in shards]
    res = run_bass_kernel_spmd(nc, in_maps, core_ids=list(range(NCORES)))
    # un-tile: y[b0+128j+p, f] = y_tiled[s, p, j, f]
    out = np.concatenate(
        [r["out"].astype(np.float32).transpose(0, 2, 1, 3).reshape(BS, F)
         for r in res.results],
        axis=0)
    out = np.ascontiguousarray(out)

    bias = inputs.get("bias")
    if bias is not None and np.any(np.asarray(bias)):
        out = out + np.asarray(bias, dtype=np.float32)[None, :]
    return out


# revision 3
# speedup vs baseline: 1.2890x; 1.0689x over previous
"""Trainium2 Bass kernel for an AQT quantized Dense layer.

Host pre-quantizes x to integer values (exact AQT rounding) and ships them
as bf16 (integers <= 127 are exact in bf16), halving the x DMA traffic and
removing all device-side quantize ops. The dequantized weight matrix
(0.5MB, bf16) is also computed on host and shipped in the matmul-ready
layout. Output is written as bf16. The device is then a pure bf16-matmul
pipeline bound by the TensorEngine: DMA in -> PE matmul -> PSUM->SBUF bf16
copy (alternating ACT/DVE) -> streamed DMA out.
"""

import numpy as np

B, D, F = 131072, 512, 512
NCORES = 8
BS = B // NCORES           # rows per core
P = 128                    # partitions
KC = D // P                # contraction chunks
SB = 1024                  # superblock: b-rows per block
NSB = BS // SB             # superblocks per core
JC = SB // P               # b-chunks of 128 rows per superblock

A_SCALE = float(np.float32(127.0 / 6.0))
EPS = 1e-6

_NC_CACHE = {}


def _build_nc():
    import concourse.bacc as bacc
    import concourse.mybir as mybir
    import concourse.tile as tile

    f32 = mybir.dt.float32
    bf16 = mybir.dt.bfloat16

    nc = bacc.Bacc("TRN2", target_bir_lowering=False, debug=False,
                   enable_asserts=False)
    x_t = nc.dram_tensor("xt", [NSB, P, KC, SB], bf16, kind="ExternalInput")
    w_t = nc.dram_tensor("wd", [KC, P, F], bf16, kind="ExternalInput")
    y_t = nc.dram_tensor("out", [NSB, P, JC, F], bf16, kind="ExternalOutput")
    x_ap, w_ap, y_ap = x_t.ap(), w_t.ap(), y_t.ap()

    with tile.TileContext(nc) as tc:
        from contextlib import ExitStack
        with ExitStack() as ctx:
            wpool = ctx.enter_context(tc.tile_pool(name="wdeq", bufs=1))
            xin = ctx.enter_context(tc.tile_pool(name="xin", bufs=6))
            yout = ctx.enter_context(tc.tile_pool(name="yout", bufs=4))
            mmps = ctx.enter_context(tc.tile_pool(name="mmps", bufs=7,
                                                  space="PSUM"))

            # dequantized weights arrive ready-to-use: [128_d, F] per chunk
            wdeq = []
            for k in range(KC):
                wd = wpool.tile([P, F], bf16, tag=f"wdeq{k}")
                nc.gpsimd.dma_start(out=wd, in_=w_ap[k])
                wdeq.append(wd)

            for s in range(NSB):
                # one 1MB load, fully contiguous (8KB per partition)
                xf = xin.tile([P, KC, SB], bf16, tag="xf")
                nc.sync.dma_start(out=xf, in_=x_ap[s])
                yf = yout.tile([P, JC, F], bf16, tag="yf")
                for j in range(JC):
                    yp = mmps.tile([P, F], f32, tag="yp")
                    for k in range(KC):
                        nc.tensor.matmul(yp,
                                         xf[:, k, j * P:(j + 1) * P],
                                         wdeq[k],
                                         start=(k == 0), stop=(k == KC - 1))
                    # PSUM -> SBUF bf16, alternating engines to stay off
                    # the PE critical path
                    if j % 2 == 0:
                        nc.scalar.copy(yf[:, j, :], yp)
                    else:
                        nc.vector.tensor_copy(yf[:, j, :], yp)
                        # stream the store per j-pair (256KB, 2KB runs)
                        with tc.high_priority():
                            nc.gpsimd.dma_start(
                                out=y_ap[s, :, j - 1:j + 1, :],
                                in_=yf[:, j - 1:j + 1, :])

    nc.compile()
    return nc


def _get_nc():
    if "nc" not in _NC_CACHE:
        _NC_CACHE["nc"] = _build_nc()
    return _NC_CACHE["nc"]


def kernel(**inputs):
    import ml_dtypes
    from concourse.bass_utils import run_bass_kernel_spmd

    x = np.asarray(inputs["x"], dtype=np.float32)
    kern = np.asarray(inputs["kernel"], dtype=np.float32)

    # AQT weight quantization + dequantization on host (0.5MB shipped)
    w_bound = np.maximum(np.abs(kern).max(axis=0, keepdims=True),
                         np.float32(EPS))
    w_scale = np.float32(127.0) / w_bound
    w_q = np.clip(np.rint(kern * w_scale), -127.0, 127.0)
    w_deq = (w_q * (w_bound / np.float32(127.0) / np.float32(A_SCALE)))
    wd = np.ascontiguousarray(
        w_deq.astype(ml_dtypes.bfloat16).reshape(KC, P, F))

    # exact AQT activation quantization on host; integer values <= 127 are
    # exactly representable in bf16
    xq = np.clip(np.rint(x * np.float32(A_SCALE)), -127.0, 127.0)
    xb = xq.astype(ml_dtypes.bfloat16)
    # packed layout: [NSB, P, KC, SB]; xtile[s, p, c, b] = x[s*SB+b, c*P+p]
    shards = [np.ascontiguousarray(
                  xb[i * BS:(i + 1) * BS].reshape(NSB, SB, KC, P)
                  .transpose(0, 3, 2, 1))
              for i in range(NCORES)]

    nc = _get_nc()
    in_maps = [{"xt": s, "wd": wd} for s in shards]
    res = run_bass_kernel_spmd(nc, in_maps, core_ids=list(range(NCORES)))
    # un-tile: y[b0+128j+p, f] = y_tiled[s, p, j, f]
    out = np.concatenate(
        [r["out"].astype(np.float32).transpose(0, 2, 1, 3).reshape(BS, F)
         for r in res.results],
        axis=0)
    out = np.ascontiguousarray(out)

    bias = inputs.get("bias")
    if bias is not None and np.any(np.asarray(bias)):
        out = out + np.asarray(bias, dtype=np.float32)[None, :]
    return out


# revision 7
# speedup vs baseline: 1.3307x; 1.0324x over previous
"""Trainium2 Bass kernel for an AQT quantized Dense layer.

Host pre-quantizes x to integer values (exact AQT rounding) and ships them
as bf16 (integers <= 127 are exact in bf16), halving the x DMA traffic and
removing all device-side quantize ops. The dequantized weight matrix
(0.5MB, bf16) is also computed on host and shipped in the matmul-ready
layout. Output is written as bf16. The device is then a pure bf16-matmul
pipeline bound by the TensorEngine: DMA in (2 queues) -> PE matmul ->
PSUM->SBUF bf16 copy (alternating ACT/DVE) -> DMA out (2 queues).
"""

import numpy as np

B, D, F = 131072, 512, 512
NCORES = 8
BS = B // NCORES           # rows per core
P = 128                    # partitions
KC = D // P                # contraction chunks
LB = 512                   # load block: b-rows per x DMA tile (4KB runs)
NLB = BS // LB             # load blocks per core
JT = LB // P               # b-chunks of 128 rows per load block
SB = 1024                  # store superblock: b-rows per y DMA (8KB runs)
NSB = BS // SB
JC = SB // P

A_SCALE = float(np.float32(127.0 / 6.0))
EPS = 1e-6

_NC_CACHE = {}


def _build_nc():
    import concourse.bacc as bacc
    import concourse.mybir as mybir
    import concourse.tile as tile

    f32 = mybir.dt.float32
    bf16 = mybir.dt.bfloat16

    nc = bacc.Bacc("TRN2", target_bir_lowering=False, debug=False,
                   enable_asserts=False)
    x_t = nc.dram_tensor("xt", [NLB, P, KC, LB], bf16, kind="ExternalInput")
    w_t = nc.dram_tensor("wd", [P, KC, F], bf16, kind="ExternalInput")
    y_t = nc.dram_tensor("out", [NSB, P, JC, F], bf16, kind="ExternalOutput")
    x_ap, w_ap, y_ap = x_t.ap(), w_t.ap(), y_t.ap()

    with tile.TileContext(nc) as tc:
        from contextlib import ExitStack
        with ExitStack() as ctx:
            wpool = ctx.enter_context(tc.tile_pool(name="wdeq", bufs=1))
            xin = ctx.enter_context(tc.tile_pool(name="xin", bufs=8))
            yout = ctx.enter_context(tc.tile_pool(name="yout", bufs=3))
            mmps = ctx.enter_context(tc.tile_pool(name="mmps", bufs=7,
                                                  space="PSUM"))

            # dequantized weights arrive ready-to-use; single DMA, 4KB runs
            wdt = wpool.tile([P, KC, F], bf16, tag="wdeq")
            nc.sync.dma_start(out=wdt, in_=w_ap)

            yf = None
            for s in range(NLB):
                # 512KB load, 4KB per-partition runs, alternating queues
                xf = xin.tile([P, KC, LB], bf16, tag="xf")
                if s % 2 == 0:
                    nc.sync.dma_start(out=xf, in_=x_ap[s])
                else:
                    nc.scalar.dma_start(out=xf, in_=x_ap[s])
                if s % 2 == 0:
                    yf = yout.tile([P, JC, F], bf16, tag="yf")
                for jj in range(JT):
                    j = (s % 2) * JT + jj
                    yp = mmps.tile([P, F], f32, tag="yp")
                    for k in range(KC):
                        nc.tensor.matmul(yp,
                                         xf[:, k, jj * P:(jj + 1) * P],
                                         wdt[:, k, :],
                                         start=(k == 0), stop=(k == KC - 1))
                    # PSUM -> SBUF bf16, alternating engines to stay off
                    # the PE critical path
                    if j % 2 == 0:
                        nc.scalar.copy(yf[:, j, :], yp)
                    else:
                        nc.vector.tensor_copy(yf[:, j, :], yp)
                        if s >= NLB - 2:
                            # final superblock: stream per-j-pair stores so
                            # the last transfer after the last copy is small
                            with tc.high_priority():
                                nc.gpsimd.dma_start(
                                    out=y_ap[s // 2, :, j - 1:j + 1, :],
                                    in_=yf[:, j - 1:j + 1, :])
                if s % 2 == 1 and s != NLB - 1:
                    # 1MB store, 8KB per-partition runs
                    with tc.high_priority():
                        nc.gpsimd.dma_start(out=y_ap[s // 2], in_=yf)

    nc.compile()
    return nc


def _get_nc():
    if "nc" not in _NC_CACHE:
        _NC_CACHE["nc"] = _build_nc()
    return _NC_CACHE["nc"]


def kernel(**inputs):
    import ml_dtypes
    from concourse.bass_utils import run_bass_kernel_spmd

    x = np.asarray(inputs["x"], dtype=np.float32)
    kern = np.asarray(inputs["kernel"], dtype=np.float32)

    # AQT weight quantization + dequantization on host (0.5MB shipped)
    w_bound = np.maximum(np.abs(kern).max(axis=0, keepdims=True),
                         np.float32(EPS))
    w_scale = np.float32(127.0) / w_bound
    w_q = np.clip(np.rint(kern * w_scale), -127.0, 127.0)
    w_deq = (w_q * (w_bound / np.float32(127.0) / np.float32(A_SCALE)))
    # layout [P, KC, F]: wd[p, k, f] = w_deq[k*128 + p, f]
    wd = np.ascontiguousarray(
        w_deq.astype(ml_dtypes.bfloat16).reshape(KC, P, F).transpose(1, 0, 2))

    # exact AQT activation quantization on host; integer values <= 127 are
    # exactly representable in bf16
    xq = np.clip(np.rint(x * np.float32(A_SCALE)), -127.0, 127.0)
    xb = xq.astype(ml_dtypes.bfloat16)
    # packed layout: [NLB, P, KC, LB]; xtile[s, p, c, b] = x[s*LB+b, c*P+p]
    shards = [np.ascontiguousarray(
                  xb[i * BS:(i + 1) * BS].reshape(NLB, LB, KC, P)
                  .transpose(0, 3, 2, 1))
              for i in range(NCORES)]

    nc = _get_nc()
    in_maps = [{"xt": s, "wd": wd} for s in shards]
    res = run_bass_kernel_spmd(nc, in_maps, core_ids=list(range(NCORES)))
    # un-tile: y[b0+128j+p, f] = y_tiled[s, p, j, f]
    out = np.concatenate(
        [r["out"].astype(np.float32).transpose(0, 2, 1, 3).reshape(BS, F)
         for r in res.results],
        axis=0)
    out = np.ascontiguousarray(out)

    bias = inputs.get("bias")
    if bias is not None and np.any(np.asarray(bias)):
        out = out + np.asarray(bias, dtype=np.float32)[None, :]
    return out


# revision 8
# speedup vs baseline: 1.3351x; 1.0033x over previous
"""Trainium2 Bass kernel for an AQT quantized Dense layer.

Host pre-quantizes x to integer values (exact AQT rounding) and ships them
as bf16 (integers <= 127 are exact in bf16), halving the x DMA traffic and
removing all device-side quantize ops. The dequantized weight matrix
(0.5MB, bf16) is also computed on host and shipped in the matmul-ready
layout. Output is written as bf16. The device is then a pure bf16-matmul
pipeline bound by the TensorEngine: DMA in (2 queues) -> PE matmul ->
PSUM->SBUF bf16 copy (alternating ACT/DVE) -> DMA out (2 queues).
"""

import numpy as np

B, D, F = 131072, 512, 512
NCORES = 8
BS = B // NCORES           # rows per core
P = 128                    # partitions
KC = D // P                # contraction chunks
LB = 512                   # load block: b-rows per x DMA tile (4KB runs)
NLB = BS // LB             # load blocks per core
JT = LB // P               # b-chunks of 128 rows per load block
SB = 1024                  # store superblock: b-rows per y DMA (8KB runs)
NSB = BS // SB
JC = SB // P

A_SCALE = float(np.float32(127.0 / 6.0))
EPS = 1e-6

_NC_CACHE = {}


def _build_nc():
    import concourse.bacc as bacc
    import concourse.mybir as mybir
    import concourse.tile as tile

    f32 = mybir.dt.float32
    bf16 = mybir.dt.bfloat16

    nc = bacc.Bacc("TRN2", target_bir_lowering=False, debug=False,
                   enable_asserts=False)
    x_t = nc.dram_tensor("xt", [NLB, P, KC, LB], bf16, kind="ExternalInput")
    w_t = nc.dram_tensor("wd", [P, KC, F], bf16, kind="ExternalInput")
    y_t = nc.dram_tensor("out", [NSB, P, JC, F], bf16, kind="ExternalOutput")
    x_ap, w_ap, y_ap = x_t.ap(), w_t.ap(), y_t.ap()

    with tile.TileContext(nc) as tc:
        from contextlib import ExitStack
        with ExitStack() as ctx:
            wpool = ctx.enter_context(tc.tile_pool(name="wdeq", bufs=1))
            xin = ctx.enter_context(tc.tile_pool(name="xin", bufs=5))
            yout = ctx.enter_context(tc.tile_pool(name="yout", bufs=3))
            mmps = ctx.enter_context(tc.tile_pool(name="mmps", bufs=7,
                                                  space="PSUM"))

            # dequantized weights arrive ready-to-use; single DMA, 4KB runs
            wdt = wpool.tile([P, KC, F], bf16, tag="wdeq")
            nc.gpsimd.dma_start(out=wdt, in_=w_ap)

            yf = None
            for s in range(NLB):
                # 512KB load, 4KB per-partition runs, alternating queues
                xf = xin.tile([P, KC, LB], bf16, tag="xf")
                if s % 2 == 0:
                    nc.sync.dma_start(out=xf, in_=x_ap[s])
                else:
                    nc.scalar.dma_start(out=xf, in_=x_ap[s])
                if s % 2 == 0:
                    yf = yout.tile([P, JC, F], bf16, tag="yf")
                for jj in range(JT):
                    j = (s % 2) * JT + jj
                    yp = mmps.tile([P, F], f32, tag="yp")
                    for k in range(KC):
                        nc.tensor.matmul(yp,
                                         xf[:, k, jj * P:(jj + 1) * P],
                                         wdt[:, k, :],
                                         start=(k == 0), stop=(k == KC - 1))
                    # PSUM -> SBUF bf16, alternating engines to stay off
                    # the PE critical path
                    if j % 2 == 0:
                        nc.scalar.copy(yf[:, j, :], yp)
                    else:
                        nc.vector.tensor_copy(yf[:, j, :], yp)
                        if s >= NLB - 2:
                            # final superblock: stream per-j-pair stores so
                            # the last transfer after the last copy is small
                            with tc.high_priority():
                                eng = nc.gpsimd if (j // 2) % 2 == 0 \
                                    else nc.scalar
                                eng.dma_start(
                                    out=y_ap[s // 2, :, j - 1:j + 1, :],
                                    in_=yf[:, j - 1:j + 1, :])
                if s % 2 == 1 and s != NLB - 1:
                    # 1MB store, 8KB per-partition runs
                    with tc.high_priority():
                        nc.gpsimd.dma_start(out=y_ap[s // 2], in_=yf)

    nc.compile()
    return nc


def _get_nc():
    if "nc" not in _NC_CACHE:
        _NC_CACHE["nc"] = _build_nc()
    return _NC_CACHE["nc"]


def kernel(**inputs):
    import ml_dtypes
    from concourse.bass_utils import run_bass_kernel_spmd

    x = np.asarray(inputs["x"], dtype=np.float32)
    kern = np.asarray(inputs["kernel"], dtype=np.float32)

    # AQT weight quantization + dequantization on host (0.5MB shipped)
    w_bound = np.maximum(np.abs(kern).max(axis=0, keepdims=True),
                         np.float32(EPS))
    w_scale = np.float32(127.0) / w_bound
    w_q = np.clip(np.rint(kern * w_scale), -127.0, 127.0)
    w_deq = (w_q * (w_bound / np.float32(127.0) / np.float32(A_SCALE)))
    # layout [P, KC, F]: wd[p, k, f] = w_deq[k*128 + p, f]
    wd = np.ascontiguousarray(
        w_deq.astype(ml_dtypes.bfloat16).reshape(KC, P, F).transpose(1, 0, 2))

    # exact AQT activation quantization on host; integer values <= 127 are
    # exactly representable in bf16
    xq = np.clip(np.rint(x * np.float32(A_SCALE)), -127.0, 127.0)
    xb = xq.astype(ml_dtypes.bfloat16)
    # packed layout: [NLB, P, KC, LB]; xtile[s, p, c, b] = x[s*LB+b, c*P+p]
    shards = [np.ascontiguousarray(
                  xb[i * BS:(i + 1) * BS].reshape(NLB, LB, KC, P)
                  .transpose(0, 3, 2, 1))
              for i in range(NCORES)]

    nc = _get_nc()
    in_maps = [{"xt": s, "wd": wd} for s in shards]
    res = run_bass_kernel_spmd(nc, in_maps, core_ids=list(range(NCORES)))
    # un-tile: y[b0+128j+p, f] = y_tiled[s, p, j, f]
    out = np.concatenate(
        [r["out"].astype(np.float32).transpose(0, 2, 1, 3).reshape(BS, F)
         for r in res.results],
        axis=0)
    out = np.ascontiguousarray(out)

    bias = inputs.get("bias")
    if bias is not None and np.any(np.asarray(bias)):
        out = out + np.asarray(bias, dtype=np.float32)[None, :]
    return out


# revision 9
# speedup vs baseline: 1.3809x; 1.0343x over previous
"""Trainium2 Bass kernel for an AQT quantized Dense layer.

Host pre-quantizes x to integer values (exact AQT rounding) and ships them
as bf16 (integers <= 127 are exact in bf16), halving the x DMA traffic and
removing all device-side quantize ops. The dequantized weight matrix
(0.5MB, bf16) is also computed on host and shipped in the matmul-ready
layout. Output is written as bf16. The device is then a pure bf16-matmul
pipeline bound by the TensorEngine: DMA in (2 queues) -> PE matmul ->
PSUM->SBUF bf16 copy (alternating ACT/DVE) -> DMA out (2 queues).
"""

import numpy as np

B, D, F = 131072, 512, 512
NCORES = 8
BS = B // NCORES           # rows per core
P = 128                    # partitions
KC = D // P                # contraction chunks
LB = 512                   # load block: b-rows per x DMA tile (4KB runs)
NLB = BS // LB             # load blocks per core
JT = LB // P               # b-chunks of 128 rows per load block
SB = 1024                  # store superblock: b-rows per y DMA (8KB runs)
NSB = BS // SB
JC = SB // P

A_SCALE = float(np.float32(127.0 / 6.0))
EPS = 1e-6

_NC_CACHE = {}


def _build_nc():
    import concourse.bacc as bacc
    import concourse.mybir as mybir
    import concourse.tile as tile

    f32 = mybir.dt.float32
    bf16 = mybir.dt.bfloat16

    nc = bacc.Bacc("TRN2", target_bir_lowering=False, debug=False,
                   enable_asserts=False)
    x_t = nc.dram_tensor("xt", [NLB, P, KC, LB], bf16, kind="ExternalInput")
    w_t = nc.dram_tensor("wd", [P, KC, F], bf16, kind="ExternalInput")
    y_t = nc.dram_tensor("out", [NSB, P, JC, F], bf16, kind="ExternalOutput")
    x_ap, w_ap, y_ap = x_t.ap(), w_t.ap(), y_t.ap()

    with tile.TileContext(nc) as tc:
        from contextlib import ExitStack
        with ExitStack() as ctx:
            wpool = ctx.enter_context(tc.tile_pool(name="wdeq", bufs=1))
            xin = ctx.enter_context(tc.tile_pool(name="xin", bufs=5))
            yout = ctx.enter_context(tc.tile_pool(name="yout", bufs=3))
            mmps = ctx.enter_context(tc.tile_pool(name="mmps", bufs=8,
                                                  space="PSUM"))

            # dequantized weights arrive ready-to-use; single DMA, 4KB runs
            # on a hardware DGE queue, issued before any x-load descriptor
            wdt = wpool.tile([P, KC, F], bf16, tag="wdeq")
            nc.scalar.dma_start(out=wdt, in_=w_ap)

            yf = None
            for s in range(NLB):
                # 512KB load, 4KB per-partition runs, alternating queues
                xf = xin.tile([P, KC, LB], bf16, tag="xf")
                if s % 2 == 0:
                    nc.sync.dma_start(out=xf, in_=x_ap[s])
                else:
                    nc.scalar.dma_start(out=xf, in_=x_ap[s])
                if s % 2 == 0:
                    yf = yout.tile([P, JC, F], bf16, tag="yf")
                for jj in range(JT):
                    j = (s % 2) * JT + jj
                    yp = mmps.tile([P, F], f32, tag="yp")
                    for k in range(KC):
                        nc.tensor.matmul(yp,
                                         xf[:, k, jj * P:(jj + 1) * P],
                                         wdt[:, k, :],
                                         start=(k == 0), stop=(k == KC - 1))
                    # PSUM -> SBUF bf16, alternating engines to stay off
                    # the PE critical path
                    if j % 2 == 0:
                        nc.scalar.copy(yf[:, j, :], yp)
                    else:
                        nc.vector.tensor_copy(yf[:, j, :], yp)
                        if s >= NLB - 2:
                            # final superblock: stream per-j-pair stores so
                            # the last transfer after the last copy is small
                            with tc.high_priority():
                                eng = nc.sync if (j // 2) % 2 == 0 \
                                    else nc.scalar
                                eng.dma_start(
                                    out=y_ap[s // 2, :, j - 1:j + 1, :],
                                    in_=yf[:, j - 1:j + 1, :])
                if s % 2 == 1 and s != NLB - 1:
                    # 1MB store, 8KB per-partition runs, alternating HW queues
                    with tc.high_priority():
                        eng = nc.sync if (s // 2) % 2 == 0 else nc.scalar
                        eng.dma_start(out=y_ap[s // 2], in_=yf)

    nc.compile()
    return nc


def _get_nc():
    if "nc" not in _NC_CACHE:
        _NC_CACHE["nc"] = _build_nc()
    return _NC_CACHE["nc"]


def kernel(**inputs):
    import ml_dtypes
    from concourse.bass_utils import run_bass_kernel_spmd

    x = np.asarray(inputs["x"], dtype=np.float32)
    kern = np.asarray(inputs["kernel"], dtype=np.float32)

    # AQT weight quantization + dequantization on host (0.5MB shipped)
    w_bound = np.maximum(np.abs(kern).max(axis=0, keepdims=True),
                         np.float32(EPS))
    w_scale = np.float32(127.0) / w_bound
    w_q = np.clip(np.rint(kern * w_scale), -127.0, 127.0)
    w_deq = (w_q * (w_bound / np.float32(127.0) / np.float32(A_SCALE)))
    # layout [P, KC, F]: wd[p, k, f] = w_deq[k*128 + p, f]
    wd = np.ascontiguousarray(
        w_deq.astype(ml_dtypes.bfloat16).reshape(KC, P, F).transpose(1, 0, 2))

    # exact AQT activation quantization on host; integer values <= 127 are
    # exactly representable in bf16
    xq = np.clip(np.rint(x * np.float32(A_SCALE)), -127.0, 127.0)
    xb = xq.astype(ml_dtypes.bfloat16)
    # packed layout: [NLB, P, KC, LB]; xtile[s, p, c, b] = x[s*LB+b, c*P+p]
    shards = [np.ascontiguousarray(
                  xb[i * BS:(i + 1) * BS].reshape(NLB, LB, KC, P)
                  .transpose(0, 3, 2, 1))
              for i in range(NCORES)]

    nc = _get_nc()
    in_maps = [{"xt": s, "wd": wd} for s in shards]
    res = run_bass_kernel_spmd(nc, in_maps, core_ids=list(range(NCORES)))
    # un-tile: y[b0+128j+p, f] = y_tiled[s, p, j, f]
    out = np.concatenate(
        [r["out"].astype(np.float32).transpose(0, 2, 1, 3).reshape(BS, F)
         for r in res.results],
        axis=0)
    out = np.ascontiguousarray(out)

    bias = inputs.get("bias")
    if bias is not None and np.any(np.asarray(bias)):
        out = out + np.asarray(bias, dtype=np.float32)[None, :]
    return out
